# revision 7
# baseline (speedup 1.0000x reference)
# GCNConv (dense adjacency, symmetric normalization) on 8 trn2 NeuronCores.
#
#   out = D^{-1/2} A D^{-1/2} (x @ W) + bias,   deg = A.sum(axis=1)
#
# Strategy (row-shard, 1D graph partition):
#   - core c owns output rows [1024c, 1024(c+1)); its shard of A is passed
#     host-side pre-transposed (adjT_c = A[rows_c, :].T, shape [8192, 1024])
#     so the contraction index j (columns of A) lands on the SBUF partition
#     axis with plain contiguous DMAs - no on-chip transposes of the shard.
#   - The shard is DMA-cast fp32->bf16 on load and kept SBUF-resident
#     (16MB), so HBM traffic is one pass over A (memory roofline).
#   - deg (row sums of A) = ones^T @ adjT on the tensor engine, accumulated
#     over the 64 j-blocks; a tiny AllGather distributes deg. The local i
#     axis is split into chunks so the first AllGather (and the SpMM work it
#     unlocks) overlaps the second half of the load.
#   - dinv = 1/sqrt(deg) via ACT sqrt + DVE reciprocal + one Newton step.
#   - h = x @ W from host-transposed xT (replicated); H' = dinv*h in bf16 is
#     the stationary operand of the SpMM:
#        outT[d, i] += sum_j H'[j, d] * adjT[j, i]
#     accumulated in PSUM over j-blocks, transposed back, scaled by local
#     dinv rows, bias added, DMA'd out.

import numpy as np

N = 8192
D = 128
NCORES = 8
P = 128


def _build(n=N, d=D, ncores=NCORES):
    from contextlib import ExitStack

    import concourse.bacc as bacc
    import concourse.masks as masks
    import concourse.mybir as mybir
    import concourse.tile as tile

    f32 = mybir.dt.float32
    bf16 = mybir.dt.bfloat16
    mult = mybir.AluOpType.mult
    add = mybir.AluOpType.add

    nb = n // P  # j-blocks (64)
    rpc = n // ncores  # rows per core (1024)
    lb = rpc // P  # local row tiles (8)
    nhalf = min(512, rpc)  # out slice width (PSUM bank limit)
    nslice = rpc // nhalf
    NCH = nslice  # i-chunks == out slices (AllGather pipelining)
    ich = rpc // NCH  # i-chunk width
    cw = nb * ich  # AT columns per chunk (chunk-major layout)
    bpd = min(8, nb)  # j-blocks per cast-DMA (8KB/partition writes)
    bpc = (P * NCH) // ncores if (P * NCH) >= ncores else 1  # j-blocks per core-chunk
    # blocks of chunk ic: b with (b % lb) in [ic*lb//NCH, (ic+1)*lb//NCH)
    lbc = lb // NCH  # local row tiles per chunk

    def chunk_of_block(b):
        return (b % lb) // lbc

    def col_in_chunk(b):
        return lbc * (b // lb) + (b % lb) - chunk_of_block(b) * lbc

    nc = bacc.Bacc("TRN2", target_bir_lowering=False, debug=False, num_devices=ncores)

    adjT = nc.dram_tensor("adjT", [n, rpc], f32, kind="ExternalInput")
    xT = nc.dram_tensor("xT", [d, n], f32, kind="ExternalInput")
    w = nc.dram_tensor("w", [d, d], f32, kind="ExternalInput")
    bias = nc.dram_tensor("bias", [d], f32, kind="ExternalInput")
    out = nc.dram_tensor("out", [rpc, d], f32, kind="ExternalOutput")

    with tile.TileContext(nc) as tc, ExitStack() as ctx:
        singles = ctx.enter_context(tc.tile_pool(name="singles", bufs=1))
        dram = ctx.enter_context(tc.tile_pool(name="dram", bufs=1, space="DRAM"))
        atp = ctx.enter_context(tc.tile_pool(name="atp", bufs=1))
        xcp = ctx.enter_context(tc.tile_pool(name="xcp", bufs=2))
        psdeg = ctx.enter_context(tc.tile_pool(name="psdeg", bufs=1, space="PSUM"))
        psh = ctx.enter_context(tc.tile_pool(name="psh", bufs=2, space="PSUM"))
        psout = ctx.enter_context(tc.tile_pool(name="psout", bufs=1, space="PSUM"))
        psmisc = ctx.enter_context(tc.tile_pool(name="psmisc", bufs=2, space="PSUM"))

        # ---- constants ----
        ident = singles.tile([P, P], f32)
        masks.make_identity(nc, ident[:])
        ones_bf = singles.tile([P, 1], bf16)
        nc.gpsimd.memset(ones_bf[:], 1.0)
        ones_row = singles.tile([1, P], f32)
        nc.gpsimd.memset(ones_row[:], 1.0)
        w_sb = singles.tile([d, d], f32)
        nc.sync.dma_start(w_sb[:], w[:, :])
        bias_row = singles.tile([1, d], f32)
        nc.sync.dma_start(bias_row[:], bias[:])
        bias_mat = singles.tile([P, d], f32)
        bm_ps = psmisc.tile([P, d], f32, tag="misc")
        nc.tensor.matmul(bm_ps[:], ones_row[:], bias_row[:])
        nc.vector.tensor_copy(bias_mat[:], bm_ps[:])

        # ---- big SBUF residents ----
        # AT chunk-major: column (ic*cw + b*ich + i) = adjT[b*P + p, ic*ich + i]
        AT = atp.tile([P, NCH * cw], bf16)
        Hp = singles.tile([P, nb * d], f32)  # h = x@W fp32, [j-part, (b d)]
        Hb = singles.tile([P, nb * d], bf16)  # H' bf16, [j-part, (b d)]

        # ---- h = x @ W  (lhsT = xT block [din, j], rhs = W [din, dout]) ----
        xch = min(1024, n)
        for c0 in range(0, n, xch):
            xc = xcp.tile([d, xch], f32)
            nc.sync.dma_start(xc[:], xT[:, c0 : c0 + xch])
            for bb in range(xch // P):
                b = c0 // P + bb
                h_ps = psh.tile([P, d], f32)
                nc.tensor.matmul(h_ps[:], xc[:, bb * P : (bb + 1) * P], w_sb[:])
                nc.scalar.copy(Hp[:, b * d : (b + 1) * d], h_ps[:])

        # ---- adjT load (cast fp32->bf16), deg accumulation, AllGather, SpMM ----
        deg_ps = [psdeg.tile([1, ich], f32, name=f"deg_ps{ic}") for ic in range(NCH)]
        out_ps = [psout.tile([P, nhalf], f32, name=f"out_ps{s}") for s in range(nslice)]
        deg_sb = singles.tile([1, rpc], f32)
        dinv_ch = [singles.tile([P, nb // NCH], f32, name=f"dinv_ch{ic}") for ic in range(NCH)]

        def rsqrt_newton(dst, deg_psum, width, tag):
            dgc = singles.tile([P, width], f32, name=f"dgc_{tag}")
            nc.vector.tensor_copy(dgc[:], deg_psum[:])
            sq = singles.tile([P, width], f32, name=f"sq_{tag}")
            nc.scalar.sqrt(sq[:], deg_psum[:])
            r0 = singles.tile([P, width], f32, name=f"r0_{tag}")
            nc.vector.reciprocal(r0[:], sq[:])
            t0 = singles.tile([P, width], f32, name=f"t0_{tag}")
            nc.vector.tensor_mul(t0[:], r0[:], r0[:])
            nc.vector.tensor_mul(t0[:], t0[:], dgc[:])
            nc.vector.tensor_scalar(t0[:], t0[:], -0.5, 1.5, mult, add)
            nc.vector.tensor_mul(dst[:], t0[:], r0[:])

        def spmm(b, s):
            nc.tensor.matmul(
                out_ps[s][:],
                Hb[:, b * d : (b + 1) * d],
                AT[:, s * cw + b * ich : s * cw + (b + 1) * ich],
                start=(b == 0),
                stop=(b == nb - 1),
                skip_group_check=True,
            )

        for ic in range(NCH):
            # load this i-chunk of every j-block; deg matmuls right behind
            for b0 in range(0, nb, bpd):
                src = adjT[b0 * P : (b0 + bpd) * P, ic * ich : (ic + 1) * ich]
                nc.gpsimd.dma_start(
                    AT[:, ic * cw + b0 * ich : ic * cw + (b0 + bpd) * ich].rearrange(
                        "p (t i) -> p t i", i=ich
                    ),
                    src.rearrange("(t p) i -> p t i", p=P),
                )
                for b in range(b0, b0 + bpd):
                    nc.tensor.matmul(
                        deg_ps[ic][:],
                        ones_bf[:],
                        AT[:, ic * cw + b * ich : ic * cw + (b + 1) * ich],
                        start=(b == 0),
                        stop=(b == nb - 1),
                    )
            # backfill this i-slice of the SpMM for already-unlocked j-blocks
            for b in range(nb):
                if chunk_of_block(b) < ic:
                    spmm(b, ic)
            # ship this chunk's deg through the AllGather
            nc.vector.tensor_copy(deg_sb[:, ic * ich : (ic + 1) * ich], deg_ps[ic][:])
            ag_in = dram.tile([ich], f32, name=f"ag_in{ic}")
            ag_out = dram.tile([ncores * ich], f32, name=f"ag_out{ic}", addr_space="Shared")
            nc.sync.dma_start(ag_in[:], deg_sb[:1, ic * ich : (ic + 1) * ich])
            nc.gpsimd.collective_compute(
                "AllGather",
                mybir.AluOpType.bypass,
                replica_groups=[list(range(ncores))],
                ins=[ag_in.opt()],
                outs=[ag_out.opt()],
            )
            nbc = nb // NCH  # j-blocks unlocked by this chunk's AllGather
            degc = singles.tile([nbc, P], f32, name=f"degc{ic}")
            nc.sync.dma_start(degc[:], ag_out[:])
            dgt_ps = psmisc.tile([P, nbc], f32, tag="misc")
            nc.tensor.transpose(dgt_ps[:], degc[:], ident[:nbc, :nbc])
            rsqrt_newton(dinv_ch[ic], dgt_ps, nbc, f"g{ic}")

            # H' + SpMM (all loaded i-slices) for the newly unlocked j-blocks
            for b in range(nb):
                if chunk_of_block(b) != ic:
                    continue
                col = col_in_chunk(b)
                nc.vector.tensor_scalar(
                    Hb[:, b * d : (b + 1) * d],
                    Hp[:, b * d : (b + 1) * d],
                    dinv_ch[ic][:, col : col + 1],
                    None,
                    mult,
                )
                for s in range(ic + 1):
                    spmm(b, s)

        # local dinv for this core's output rows, [p, r] layout
        dloc_ps = psmisc.tile([P, lb], f32, tag="misc")
        for r in range(lb):
            nc.tensor.transpose(
                dloc_ps[:, r : r + 1], deg_sb[:1, r * P : (r + 1) * P], ident[:1, :1]
            )
        dinvl = singles.tile([P, lb], f32)
        rsqrt_newton(dinvl, dloc_ps, lb, "l")

        # ---- finalize: transpose back, scale by dinv rows, add bias ----
        outT_sb = singles.tile([P, rpc], f32)
        for s in range(nslice):
            nc.scalar.copy(outT_sb[:, s * nhalf : (s + 1) * nhalf], out_ps[s][:])
        out_sb = singles.tile([P, lb * d], f32)
        for r in range(lb):
            ob_ps = psmisc.tile([P, d], f32, tag="misc")
            nc.tensor.transpose(ob_ps[:], outT_sb[:, r * P : (r + 1) * P], ident[:])
            nc.vector.tensor_scalar(
                out_sb[:, r * d : (r + 1) * d], ob_ps[:], dinvl[:, r : r + 1], None, mult
            )
            nc.vector.tensor_add(
                out_sb[:, r * d : (r + 1) * d], out_sb[:, r * d : (r + 1) * d], bias_mat[:]
            )
        nc.sync.dma_start(
            out.ap().rearrange("(r p) d -> p r d", p=P),
            out_sb[:].rearrange("p (r d) -> p r d", d=d),
        )

    nc.compile()
    return nc


_NC_CACHE = {}


def _get_nc(n=N, d=D, ncores=NCORES):
    key = (n, d, ncores)
    if key not in _NC_CACHE:
        _NC_CACHE[key] = _build(n, d, ncores)
    return _NC_CACHE[key]


def run(x, adj, weight, bias, n=N, d=D, ncores=NCORES, trace=False):
    from concourse import bass_utils

    x = np.ascontiguousarray(np.asarray(x, dtype=np.float32))
    adj = np.ascontiguousarray(np.asarray(adj, dtype=np.float32))
    weight = np.ascontiguousarray(np.asarray(weight, dtype=np.float32))
    bias = np.ascontiguousarray(np.asarray(bias, dtype=np.float32))

    rpc = n // ncores
    xT = np.ascontiguousarray(x.T)
    in_maps = []
    for c in range(ncores):
        adjT_c = np.ascontiguousarray(adj[c * rpc : (c + 1) * rpc, :].T)
        in_maps.append({"adjT": adjT_c, "xT": xT, "w": weight, "bias": bias})

    nc = _get_nc(n, d, ncores)
    res = bass_utils.run_bass_kernel_spmd(
        nc, in_maps, core_ids=list(range(ncores)), trace=trace
    )
    out = np.concatenate([r["out"] for r in res.results], axis=0)
    return out, res


def kernel(x, adj, weight, bias):
    out, _ = run(x, adj, weight, bias)
    return out


# revision 8
# speedup vs baseline: 1.0805x; 1.0805x over previous
# GCNConv (dense adjacency, symmetric normalization) on 8 trn2 NeuronCores.
#
#   out = D^{-1/2} A D^{-1/2} (x @ W) + bias,   deg = A.sum(axis=1)
#
# Strategy (row-shard, 1D graph partition):
#   - core c owns output rows [1024c, 1024(c+1)); its shard of A is passed
#     host-side pre-transposed (adjT_c = A[rows_c, :].T, shape [8192, 1024])
#     so the contraction index j (columns of A) lands on the SBUF partition
#     axis with plain contiguous DMAs - no on-chip transposes of the shard.
#   - The shard streams in fp32 over HWDGE, is cast to bf16 on the vector
#     engine, and stays SBUF-resident (16MB): one HBM pass over A.
#   - deg (row sums of A) = ones^T @ adjT on the tensor engine, accumulated
#     over the 64 j-blocks; a tiny AllGather distributes deg. The local i
#     axis is chunked so the first AllGather overlaps the rest of the load.
#   - dinv = 1/sqrt(deg) via ACT sqrt + DVE reciprocal + one Newton step.
#   - h = x @ W from host-transposed xT (replicated); H' = dinv*h in bf16 is
#     the stationary operand of the SpMM:
#        outT[d, i] += sum_j H'[j, d] * adjT[j, i]
#     accumulated in PSUM over j-blocks, transposed back, scaled by local
#     dinv rows, bias added, DMA'd out.

import numpy as np

N = 8192
D = 128
NCORES = 8
P = 128


def _build(n=N, d=D, ncores=NCORES):
    from contextlib import ExitStack

    import concourse.bacc as bacc
    import concourse.masks as masks
    import concourse.mybir as mybir
    import concourse.tile as tile

    f32 = mybir.dt.float32
    bf16 = mybir.dt.bfloat16
    mult = mybir.AluOpType.mult
    add = mybir.AluOpType.add

    nb = n // P  # j-blocks (64)
    rpc = n // ncores  # rows per core (1024)
    lb = rpc // P  # local row tiles (8)
    nhalf = min(512, rpc)  # out slice width (PSUM bank limit)
    nslice = rpc // nhalf
    NCH = nslice  # i-chunks == out slices (AllGather pipelining)
    ich = rpc // NCH  # i-chunk width (512)
    cw = nb * ich  # AT columns per chunk (chunk-major layout)
    bpd = min(8, nb)  # j-blocks per load DMA
    lbc = lb // NCH  # local row tiles per chunk

    def chunk_of_block(b):
        return (b % lb) // lbc

    def col_in_chunk(b):
        return lbc * (b // lb) + (b % lb) - chunk_of_block(b) * lbc

    nc = bacc.Bacc("TRN2", target_bir_lowering=False, debug=False, num_devices=ncores)

    adjT = nc.dram_tensor("adjT", [n, rpc], f32, kind="ExternalInput")
    xT = nc.dram_tensor("xT", [d, n], f32, kind="ExternalInput")
    w = nc.dram_tensor("w", [d, d], f32, kind="ExternalInput")
    bias = nc.dram_tensor("bias", [d], f32, kind="ExternalInput")
    out = nc.dram_tensor("out", [rpc, d], f32, kind="ExternalOutput")

    with tile.TileContext(nc) as tc, ExitStack() as ctx:
        singles = ctx.enter_context(tc.tile_pool(name="singles", bufs=1))
        dram = ctx.enter_context(tc.tile_pool(name="dram", bufs=1, space="DRAM"))
        atp = ctx.enter_context(tc.tile_pool(name="atp", bufs=1))
        stp = ctx.enter_context(tc.tile_pool(name="stp", bufs=2))
        xcp = ctx.enter_context(tc.tile_pool(name="xcp", bufs=2))
        psdeg = ctx.enter_context(tc.tile_pool(name="psdeg", bufs=1, space="PSUM"))
        psh = ctx.enter_context(tc.tile_pool(name="psh", bufs=2, space="PSUM"))
        psout = ctx.enter_context(tc.tile_pool(name="psout", bufs=1, space="PSUM"))
        psmisc = ctx.enter_context(tc.tile_pool(name="psmisc", bufs=2, space="PSUM"))

        # ---- constants ----
        ident = singles.tile([P, P], f32)
        masks.make_identity(nc, ident[:])
        ones_bf = singles.tile([P, 1], bf16)
        nc.gpsimd.memset(ones_bf[:], 1.0)
        ones_row = singles.tile([1, P], f32)
        nc.gpsimd.memset(ones_row[:], 1.0)
        w_sb = singles.tile([d, d], f32)
        nc.gpsimd.dma_start(w_sb[:], w[:, :])
        bias_row = singles.tile([1, d], f32)
        nc.gpsimd.dma_start(bias_row[:], bias[:])
        bias_mat = singles.tile([P, d], f32)
        bm_ps = psmisc.tile([P, d], f32, tag="misc")
        nc.tensor.matmul(bm_ps[:], ones_row[:], bias_row[:])
        nc.vector.tensor_copy(bias_mat[:], bm_ps[:])

        # ---- big SBUF residents ----
        # AT chunk-major: column (ic*cw + b*ich + i) = adjT[b*P + p, ic*ich + i]
        AT = atp.tile([P, NCH * cw], bf16)
        Hb = singles.tile([P, nb * d], bf16)  # h then H' (in place), [j, (b d)]

        # ---- h = x @ W  (lhsT = xT block [din, j], rhs = W [din, dout]) ----
        xch = min(1024, n)
        for c0 in range(0, n, xch):
            xc = xcp.tile([d, xch], f32)
            nc.gpsimd.dma_start(xc[:], xT[:, c0 : c0 + xch])
            for bb in range(xch // P):
                b = c0 // P + bb
                h_ps = psh.tile([P, d], f32)
                nc.tensor.matmul(h_ps[:], xc[:, bb * P : (bb + 1) * P], w_sb[:])
                nc.scalar.copy(Hb[:, b * d : (b + 1) * d], h_ps[:])

        deg_ps = [psdeg.tile([1, ich], f32, name=f"deg_ps{ic}") for ic in range(NCH)]
        out_ps = [psout.tile([P, nhalf], f32, name=f"out_ps{s}") for s in range(nslice)]
        deg_sb = singles.tile([1, rpc], f32)
        dinv_ch = [singles.tile([P, nb // NCH], f32, name=f"dinv_ch{ic}") for ic in range(NCH)]
        ag_outs = []

        # ---- phase A: stream adjT in (fp32 -> bf16), deg, AllGathers ----
        for ic in range(NCH):
            for b0 in range(0, nb, bpd):
                stage = stp.tile([P, bpd * ich], f32)
                src = adjT[b0 * P : (b0 + bpd) * P, ic * ich : (ic + 1) * ich]
                eng = nc.sync if (b0 // bpd) % 2 == 0 else nc.scalar
                eng.dma_start(
                    stage[:].rearrange("p (t i) -> p t i", i=ich),
                    src.rearrange("(t p) i -> p t i", p=P),
                )
                nc.vector.tensor_scalar(
                    AT[:, ic * cw + b0 * ich : ic * cw + (b0 + bpd) * ich],
                    stage[:],
                    1.0,
                    None,
                    mult,
                )
                for b in range(b0, b0 + bpd):
                    nc.tensor.matmul(
                        deg_ps[ic][:],
                        ones_bf[:],
                        AT[:, ic * cw + b * ich : ic * cw + (b + 1) * ich],
                        start=(b == 0),
                        stop=(b == nb - 1),
                    )
            # ship this chunk's deg through the AllGather
            nc.vector.tensor_copy(deg_sb[:, ic * ich : (ic + 1) * ich], deg_ps[ic][:])
            ag_in = dram.tile([ich], f32, name=f"ag_in{ic}")
            ag_out = dram.tile([ncores * ich], f32, name=f"ag_out{ic}", addr_space="Shared")
            nc.sync.dma_start(ag_in[:], deg_sb[:1, ic * ich : (ic + 1) * ich])
            nc.gpsimd.collective_compute(
                "AllGather",
                mybir.AluOpType.bypass,
                replica_groups=[list(range(ncores))],
                ins=[ag_in.opt()],
                outs=[ag_out.opt()],
            )
            ag_outs.append(ag_out)

        def rsqrt_newton(dst, deg_psum, width, tag):
            dgc = singles.tile([P, width], f32, name=f"dgc_{tag}")
            nc.vector.tensor_copy(dgc[:], deg_psum[:])
            sq = singles.tile([P, width], f32, name=f"sq_{tag}")
            nc.scalar.sqrt(sq[:], deg_psum[:])
            r0 = singles.tile([P, width], f32, name=f"r0_{tag}")
            nc.vector.reciprocal(r0[:], sq[:])
            t0 = singles.tile([P, width], f32, name=f"t0_{tag}")
            nc.vector.tensor_mul(t0[:], r0[:], r0[:])
            nc.vector.tensor_mul(t0[:], t0[:], dgc[:])
            nc.vector.tensor_scalar(t0[:], t0[:], -0.5, 1.5, mult, add)
            nc.vector.tensor_mul(dst[:], t0[:], r0[:])

        # ---- phase B: per chunk, dinv + H' + SpMM over all loaded slices ----
        for ic in range(NCH):
            nbc = nb // NCH
            degc = singles.tile([nbc, P], f32, name=f"degc{ic}")
            nc.sync.dma_start(degc[:], ag_outs[ic][:])
            dgt_ps = psmisc.tile([P, nbc], f32, tag="misc")
            nc.tensor.transpose(dgt_ps[:], degc[:], ident[:nbc, :nbc])
            rsqrt_newton(dinv_ch[ic], dgt_ps, nbc, f"g{ic}")
            for b in range(nb):
                if chunk_of_block(b) != ic:
                    continue
                col = col_in_chunk(b)
                nc.vector.tensor_scalar(
                    Hb[:, b * d : (b + 1) * d],
                    Hb[:, b * d : (b + 1) * d],
                    dinv_ch[ic][:, col : col + 1],
                    None,
                    mult,
                )
                for s in range(nslice):
                    nc.tensor.matmul(
                        out_ps[s][:],
                        Hb[:, b * d : (b + 1) * d],
                        AT[:, s * cw + b * ich : s * cw + (b + 1) * ich],
                        start=(b == 0),
                        stop=(b == nb - 1),
                        skip_group_check=True,
                    )

        # local dinv for this core's output rows, [p, r] layout
        dloc_ps = psmisc.tile([P, lb], f32, tag="misc")
        for r in range(lb):
            nc.tensor.transpose(
                dloc_ps[:, r : r + 1], deg_sb[:1, r * P : (r + 1) * P], ident[:1, :1]
            )
        dinvl = singles.tile([P, lb], f32)
        rsqrt_newton(dinvl, dloc_ps, lb, "l")

        # ---- finalize: transpose back, scale by dinv rows, add bias ----
        outT_sb = singles.tile([P, rpc], f32)
        for s in range(nslice):
            nc.scalar.copy(outT_sb[:, s * nhalf : (s + 1) * nhalf], out_ps[s][:])
        out_sb = singles.tile([P, lb * d], f32)
        for r in range(lb):
            ob_ps = psmisc.tile([P, d], f32, tag="misc")
            nc.tensor.transpose(ob_ps[:], outT_sb[:, r * P : (r + 1) * P], ident[:])
            nc.vector.tensor_scalar(
                out_sb[:, r * d : (r + 1) * d], ob_ps[:], dinvl[:, r : r + 1], None, mult
            )
            nc.vector.tensor_add(
                out_sb[:, r * d : (r + 1) * d], out_sb[:, r * d : (r + 1) * d], bias_mat[:]
            )
        nc.sync.dma_start(
            out.ap().rearrange("(r p) d -> p r d", p=P),
            out_sb[:].rearrange("p (r d) -> p r d", d=d),
        )

    nc.compile()
    return nc


_NC_CACHE = {}


def _get_nc(n=N, d=D, ncores=NCORES):
    key = (n, d, ncores)
    if key not in _NC_CACHE:
        _NC_CACHE[key] = _build(n, d, ncores)
    return _NC_CACHE[key]


def run(x, adj, weight, bias, n=N, d=D, ncores=NCORES, trace=False):
    from concourse import bass_utils

    x = np.ascontiguousarray(np.asarray(x, dtype=np.float32))
    adj = np.ascontiguousarray(np.asarray(adj, dtype=np.float32))
    weight = np.ascontiguousarray(np.asarray(weight, dtype=np.float32))
    bias = np.ascontiguousarray(np.asarray(bias, dtype=np.float32))

    rpc = n // ncores
    xT = np.ascontiguousarray(x.T)
    in_maps = []
    for c in range(ncores):
        adjT_c = np.ascontiguousarray(adj[c * rpc : (c + 1) * rpc, :].T)
        in_maps.append({"adjT": adjT_c, "xT": xT, "w": weight, "bias": bias})

    nc = _get_nc(n, d, ncores)
    res = bass_utils.run_bass_kernel_spmd(
        nc, in_maps, core_ids=list(range(ncores)), trace=trace
    )
    out = np.concatenate([r["out"] for r in res.results], axis=0)
    return out, res


def kernel(x, adj, weight, bias):
    out, _ = run(x, adj, weight, bias)
    return out
